# revision 2
# baseline (speedup 1.0000x reference)
"""Dilated-attention (segmented FlashMHA) for Trainium2, 8-core data parallel.

Problem (hardcoded): x [2, 8192, 1024], SEGMENT=2048, DILATION=2, 16 heads.
Each (batch, segment) pair is an independent attention problem over the
L = 1024 dilated tokens; there are exactly B * n_seg = 2 * 4 = 8 of them,
one per NeuronCore.  Weights are replicated.

Per-core Bass kernel (all matmuls float32r = full PE rate at N>=256):
  xsT  = transpose(xs)                     PE transpose, 64 128x128 blocks
  qkT  = Wqkv[:, :2048].T @ xsT  (+bias)   q/k kept transposed [dim, token]
  v    = xs @ Wqkv[:, 2048:]    (+bias)    natural [token, dim], stored
                                           head-blocked with a ones column
                                           appended per head (v_aug)
  per head:  sT = k.q (transposed scores), eT = exp(sT/8) via ACT,
             ctxT_aug = sum_ck v_aug.T-contract @ eT  ([65, lq]; row 64 is
             the softmax denominator thanks to the ones column),
             ctxT = ctxT_aug[0:64] / denom (DVE + gpsimd partition bcast)
  out  = ctxT.T-contract @ Wout + bout     natural layout, DMA to HBM
"""

from contextlib import ExitStack

import numpy as np

from concourse import bacc, bass_utils, mybir, tile
from concourse._compat import with_exitstack
from concourse.masks import make_identity

F32 = mybir.dt.float32
F32R = mybir.dt.float32r
AF = mybir.ActivationFunctionType

B = 2
S = 8192
D = 1024
SEGMENT = 2048
DILATION = 2
N_SEG = S // SEGMENT          # 4
L = SEGMENT // DILATION       # 1024 tokens per (b, seg)
H = 16
HD = 64
NQK = 2048
SCALE = 0.125                 # 1 / sqrt(HD)
N_CORES = 8

_CACHE = {}


def _build(n_cores=N_CORES):
    nc = bacc.Bacc("TRN2", debug=False, num_devices=n_cores)

    xs_d = nc.dram_tensor("xs", (L, D), F32, kind="ExternalInput")
    wqkv_d = nc.dram_tensor("Wqkv", (D, 3 * D), F32, kind="ExternalInput")
    bqkv_d = nc.dram_tensor("bqkv", (3 * D,), F32, kind="ExternalInput")
    wout_d = nc.dram_tensor("Wout", (D, D), F32, kind="ExternalInput")
    bout_d = nc.dram_tensor("bout", (D,), F32, kind="ExternalInput")
    out_d = nc.dram_tensor("out", (L, D), F32, kind="ExternalOutput")

    with tile.TileContext(nc) as tc:
        _emit(tc, out_d.ap(), xs_d.ap(), wqkv_d.ap(), bqkv_d.ap(),
              wout_d.ap(), bout_d.ap())
    nc.compile()
    return nc


@with_exitstack
def _emit(ctx: ExitStack, tc, out, xs, wqkv, bqkv, wout, bout):
    nc = tc.nc

    const_p = ctx.enter_context(tc.tile_pool(name="const", bufs=1))
    ctxT_p = ctx.enter_context(tc.tile_pool(name="ctxT", bufs=8))

    identity = const_p.tile([128, 128], F32)
    make_identity(nc, identity[:])

    bqk = const_p.tile([128, 16], F32)
    nc.sync.dma_start(out=bqk[:], in_=bqkv[0:NQK].rearrange("(c p) -> p c", p=128))
    bv_row = const_p.tile([1, D], F32R)
    nc.sync.dma_start(out=bv_row[:], in_=bqkv[NQK:3 * D].unsqueeze(0).bitcast(F32R))
    bout_row = const_p.tile([1, D], F32R)
    nc.sync.dma_start(out=bout_row[:], in_=bout.unsqueeze(0).bitcast(F32R))
    # memset cannot produce float32r directly; stage in f32 and copy
    ones_f32 = const_p.tile([128, 16], F32)
    nc.vector.memset(ones_f32[:], 1.0)
    ones_row_f32 = const_p.tile([1, 128], F32)
    nc.vector.memset(ones_row_f32[:], 1.0)
    ones_col = const_p.tile([1, 128], F32R)
    nc.vector.tensor_copy(out=ones_col[:], in_=ones_row_f32[:])

    ctxT = [ctxT_p.tile([128, L], F32R, tag="ctxT", name=f"ctxT{i}")
            for i in range(8)]

    with tc.tile_pool(name="qkT", bufs=16) as qkT_p, \
         tc.tile_pool(name="vaug", bufs=8) as vaug_p:

        qkT = [qkT_p.tile([128, L], F32R, tag="qkT", name=f"qkT{i}")
               for i in range(16)]
        vaug = [vaug_p.tile([128, H * (HD + 1)], F32R, tag="vaug",
                            name=f"vaug{i}") for i in range(8)]

        with tc.tile_pool(name="xsT", bufs=8) as xsT_p:
            xsT = [xsT_p.tile([128, L], F32R, tag="xsT", name=f"xsT{i}")
                   for i in range(8)]

            # ---- phase 0: load xs, build xsT -----------------------------
            with tc.tile_pool(name="xs_nat", bufs=8) as xsn_p, \
                 tc.tile_pool(name="tp_ps", bufs=4, space="PSUM") as tp_ps:
                xs_nat = []
                for r in range(8):
                    t = xsn_p.tile([128, D], F32, tag="xsn", name=f"xsn{r}")
                    nc.sync.dma_start(out=t[:], in_=xs[r * 128:(r + 1) * 128, :])
                    xs_nat.append(t)

                for c in range(8):
                    for g in range(2):
                        pt = tp_ps.tile([128, 512], F32, tag="tp", name="tp")
                        for k in range(4):
                            r = g * 4 + k
                            nc.tensor.transpose(
                                pt[:, k * 128:(k + 1) * 128],
                                xs_nat[r][:, c * 128:(c + 1) * 128],
                                identity[:],
                            )
                        nc.vector.tensor_copy(
                            out=xsT[c][:, g * 512:(g + 1) * 512], in_=pt[:])

            # ---- phase 1a: qkT = Wqkv[:, :2048].T @ xsT (+bias) ----------
            with tc.tile_pool(name="wcol", bufs=3) as wc_p, \
                 tc.tile_pool(name="qk_ps", bufs=4, space="PSUM") as qk_ps:
                for m in range(16):
                    wcol = wc_p.tile([128, 8, 128], F32R, tag="w", name="wcol")
                    nc.sync.dma_start(
                        out=wcol[:],
                        in_=wqkv[:, m * 128:(m + 1) * 128]
                        .rearrange("(r p) m -> p r m", p=128).bitcast(F32R),
                    )
                    for half in range(2):
                        ps = qk_ps.tile([128, 512], F32, tag="proj", name="ps")
                        for r in range(8):
                            nc.tensor.matmul(
                                ps[:], wcol[:, r, :],
                                xsT[r][:, half * 512:(half + 1) * 512],
                                start=(r == 0), stop=(r == 7),
                            )
                        nc.scalar.activation(
                            out=qkT[m][:, half * 512:(half + 1) * 512],
                            in_=ps[:],
                            func=AF.Identity, bias=bqk[:, m:m + 1], scale=1.0,
                        )

            # ---- phase 1b: v = xs @ Wv (+bias), head-blocked -------------
            for l in range(8):
                dst = vaug[l][:].rearrange("p (h e) -> p h e", e=HD + 1)
                nc.vector.tensor_copy(out=dst[:, :, HD:HD + 1],
                                      in_=ones_f32[:].unsqueeze(2))
            with tc.tile_pool(name="wv", bufs=2) as wv_p, \
                 tc.tile_pool(name="v_ps", bufs=4, space="PSUM") as v_ps:
                for q in range(4):
                    wv = wv_p.tile([128, 8, 256], F32R, tag="wv", name="wv")
                    nc.sync.dma_start(
                        out=wv[:],
                        in_=wqkv[:, NQK + q * 256:NQK + (q + 1) * 256]
                        .rearrange("(r p) n -> p r n", p=128).bitcast(F32R),
                    )
                    for l in range(8):
                        ps = v_ps.tile([128, 256], F32, tag="vproj", name="vps")
                        for r in range(8):
                            nc.tensor.matmul(
                                ps[:], xsT[r][:, l * 128:(l + 1) * 128],
                                wv[:, r, :],
                                start=(r == 0), stop=False,
                            )
                        nc.tensor.matmul(
                            ps[:], ones_col[:],
                            bv_row[:, q * 256:(q + 1) * 256],
                            start=False, stop=True,
                        )
                        dst = vaug[l][:].rearrange("p (h e) -> p h e", e=HD + 1)
                        nc.vector.tensor_copy(
                            out=dst[:, q * 4:(q + 1) * 4, 0:HD],
                            in_=ps[:].rearrange("p (h e) -> p h e", e=HD),
                        )

        # ---- phase 2: attention per head ---------------------------------
        with tc.tile_pool(name="expT", bufs=4) as exp_p, \
             tc.tile_pool(name="srow", bufs=4) as srow_p, \
             tc.tile_pool(name="rbc", bufs=2) as rbc_p, \
             tc.tile_pool(name="s_ps", bufs=2, space="PSUM") as s_ps, \
             tc.tile_pool(name="c_ps", bufs=2, space="PSUM") as c_ps:

            for h in range(H):
                qt = qkT[h // 2]
                kt = qkT[8 + h // 2]
                po = (h % 2) * HD
                cps = c_ps.tile([128, L], F32, tag="cps", name="cps")
                for c in range(8):
                    sps = s_ps.tile([128, L], F32, tag="sps", name="sps")
                    for half in range(2):
                        nc.tensor.matmul(
                            sps[:, half * 512:(half + 1) * 512],
                            kt[po:po + HD, c * 128:(c + 1) * 128],
                            qt[po:po + HD, half * 512:(half + 1) * 512],
                            start=True, stop=True,
                        )
                    et = exp_p.tile([128, L], F32R, tag="expT", name="et")
                    nc.scalar.activation(out=et[:], in_=sps[:], func=AF.Exp,
                                         scale=SCALE)
                    for half in range(2):
                        nc.tensor.matmul(
                            cps[0:HD + 1, half * 512:(half + 1) * 512],
                            vaug[c][:, h * (HD + 1):(h + 1) * (HD + 1)],
                            et[:, half * 512:(half + 1) * 512],
                            start=(c == 0), stop=(c == 7),
                        )
                rec = srow_p.tile([1, L], F32, tag="srow", name="rec")
                nc.vector.reciprocal(out=rec[:], in_=cps[HD:HD + 1, :])
                rbc = rbc_p.tile([HD, L], F32, tag="rbc", name="rbc")
                nc.gpsimd.partition_broadcast(rbc[:], rec[:])
                nc.vector.tensor_mul(
                    ctxT[h // 2][po:po + HD, :], cps[0:HD, :], rbc[:])

    # ---- phase 3: out = ctxT.T-contract @ Wout + bout --------------------
    with tc.tile_pool(name="wout", bufs=8) as wo_p, \
         tc.tile_pool(name="o_sb", bufs=4) as o_sb, \
         tc.tile_pool(name="o_ps", bufs=4, space="PSUM") as o_ps:
        wo = []
        for r in range(8):
            t = wo_p.tile([128, D], F32R, tag="wo", name=f"wo{r}")
            nc.sync.dma_start(
                out=t[:], in_=wout[r * 128:(r + 1) * 128, :].bitcast(F32R))
            wo.append(t)
        for l in range(8):
            for half in range(2):
                ps = o_ps.tile([128, 512], F32, tag="ops", name="ops")
                for r in range(8):
                    nc.tensor.matmul(
                        ps[:], ctxT[r][:, l * 128:(l + 1) * 128],
                        wo[r][:, half * 512:(half + 1) * 512],
                        start=(r == 0), stop=False,
                    )
                nc.tensor.matmul(
                    ps[:], ones_col[:],
                    bout_row[:, half * 512:(half + 1) * 512],
                    start=False, stop=True,
                )
                osb = o_sb.tile([128, 512], F32, tag="osb", name="osb")
                nc.vector.tensor_copy(out=osb[:], in_=ps[:])
                nc.sync.dma_start(
                    out=out[l * 128:(l + 1) * 128,
                            half * 512:(half + 1) * 512],
                    in_=osb[:],
                )


def get_nc():
    if "nc" not in _CACHE:
        _CACHE["nc"] = _build()
    return _CACHE["nc"]


def make_in_maps(x, Wqkv, bqkv, Wout, bout):
    """Shard: core i -> (batch i//N_SEG, segment i%N_SEG), dilated tokens."""
    x = np.asarray(x, dtype=np.float32)
    Wqkv = np.ascontiguousarray(np.asarray(Wqkv, dtype=np.float32))
    bqkv = np.ascontiguousarray(np.asarray(bqkv, dtype=np.float32))
    Wout = np.ascontiguousarray(np.asarray(Wout, dtype=np.float32))
    bout = np.ascontiguousarray(np.asarray(bout, dtype=np.float32))
    in_maps = []
    for i in range(N_CORES):
        b, seg = divmod(i, N_SEG)
        xs = np.ascontiguousarray(
            x[b, seg * SEGMENT:(seg + 1) * SEGMENT:DILATION, :])
        in_maps.append({"xs": xs, "Wqkv": Wqkv, "bqkv": bqkv,
                        "Wout": Wout, "bout": bout})
    return in_maps


def unshard(results):
    out = np.empty((B, N_SEG * L, D), dtype=np.float32)
    for i in range(N_CORES):
        b, seg = divmod(i, N_SEG)
        out[b, seg * L:(seg + 1) * L, :] = results[i]["out"]
    return out


def kernel(x, Wqkv, bqkv, Wout, bout):
    nc = get_nc()
    in_maps = make_in_maps(x, Wqkv, bqkv, Wout, bout)
    res = bass_utils.run_bass_kernel_spmd(nc, in_maps,
                                          core_ids=list(range(N_CORES)))
    return unshard(res.results)
